# revision 1
# baseline (speedup 1.0000x reference)
"""Trainium2 Bass kernel for sparse (top-k=64) talking-heads causal attention.

Sharding: queries striped across 8 cores — core c owns query blocks {c, 15-c}
(128 rows each) for both batches and all 16 heads, so the talking-heads mix
(couples all heads at fixed (b,i,j)) stays core-local and causal work is
balanced. Uniform SPMD instruction stream: every core processes one 1152-wide
tile (low block) and one 2176-wide tile (high block); true causal widths are
enforced by per-core additive masks (host data).

Talking-heads fold: mixed[b,g,i,j] = sum_{h,d} (pre[h,g]*SCALE*q[b,h,i,d]) * K[b,j,(h,d)]
 -> one 1024-contraction matmul per output head g with per-head-scaled q'.
Memory k/v appended at j in [2048,2064) (j order is irrelevant: top-k /
softmax / AV are permutation invariant).

Top-64 threshold: 12-step binary search on exact counts; count(d >= t) is one
DVE tensor_scalar(is_ge, accum_out) op per tile per step; final t = lo bracket
(keeps >= 64). Rows with <= 64 valid entries converge to t=-16 => keep all.
"""
import sys
import numpy as np
import ml_dtypes

sys.path.insert(0, "/opt/trn_rl_repo")

B, N, DIM = 2, 2048, 1024
H, DH = 16, 64
M = 16
TOPK = 64
SCALE = DH ** -0.5
NEGF = -3.0e38
BF = ml_dtypes.bfloat16

WA, WB = 1152, 2176
NBA, NBB = WA // 128, WB // 128
NSEARCH = 12
BRLO, BRW = -16.0, 32.0

_CACHE = {}
import os
DBG = set(os.environ.get("KDBG", "").split(","))


def _build_nc():
    import concourse.mybir as mybir
    from concourse import bacc, tile

    fp32 = mybir.dt.float32
    bf16 = mybir.dt.bfloat16
    Alu = mybir.AluOpType
    ActF = mybir.ActivationFunctionType

    nc = bacc.Bacc(None, target_bir_lowering=False)

    d_xtq = nc.dram_tensor("xtq", [DIM, 512], bf16, kind="ExternalInput")
    d_xt = nc.dram_tensor("xt", [DIM, 2 * N], bf16, kind="ExternalInput")
    d_wq = nc.dram_tensor("wq", [DIM, DIM], bf16, kind="ExternalInput")
    d_wkv = nc.dram_tensor("wkv", [DIM, 2 * DIM], bf16, kind="ExternalInput")
    d_wo = nc.dram_tensor("wo", [DIM, DIM], bf16, kind="ExternalInput")
    d_bo = nc.dram_tensor("bob", [128, DIM], bf16, kind="ExternalInput")
    d_sq = nc.dram_tensor("sq", [128, 8 * H], fp32, kind="ExternalInput")
    d_mA = nc.dram_tensor("maska", [128, WA], bf16, kind="ExternalInput")
    d_mB = nc.dram_tensor("maskb", [128, WB], bf16, kind="ExternalInput")
    d_mkT = nc.dram_tensor("memkt", [DIM, M], bf16, kind="ExternalInput")
    d_mv = nc.dram_tensor("memv", [M, DIM], bf16, kind="ExternalInput")
    d_y = nc.dram_tensor("y", [512, DIM], fp32, kind="ExternalOutput")

    with tile.TileContext(nc) as tc:
        with tc.tile_pool(name="persist", bufs=1) as pp, \
             tc.tile_pool(name="psA", bufs=2, space="PSUM") as psA, \
             tc.tile_pool(name="psB", bufs=2, space="PSUM") as psB, \
             tc.tile_pool(name="psO", bufs=2, space="PSUM") as psO:

            kt_bf = pp.tile([128, 8, 2 * WB], bf16, tag="kt")   # [slice, b*WB + j]
            v_bf = pp.tile([128, 2 * 17, DIM], bf16, tag="v")   # [b*17 + jblk, (h d)]
            qt_bf = pp.tile([128, 8, 512], bf16, tag="qt")

            nc.vector.memset(kt_bf[:], 0.0)
            nc.vector.memset(v_bf[:], 0.0)

            # ---------- stages 0-1: K^T, V ----------
            with tc.tile_pool(name="wA", bufs=1) as wA, \
                 tc.tile_pool(name="xs", bufs=2) as xs:
                wkv_bf = wA.tile([128, 8, 2 * DIM], bf16, tag="wkv")
                nc.sync.dma_start(
                    wkv_bf[:], d_wkv.rearrange("(s p) t -> p s t", p=128))
                # mem keys / values
                stgk = wA.tile([128, 8 * M], bf16, tag="stgk")
                for s in range(8):
                    nc.sync.dma_start(
                        stgk[:, s * M:(s + 1) * M], d_mkT[s * 128:(s + 1) * 128, :])
                for b in range(2):
                    for s in range(8):
                        nc.vector.tensor_copy(
                            kt_bf[:, s, b * WB + N:b * WB + N + M],
                            stgk[:, s * M:(s + 1) * M])
                stgv = wA.tile([128, DIM], bf16, tag="stgv")
                nc.sync.dma_start(stgv[:M, :], d_mv[:, :])
                for b in range(2):
                    nc.vector.tensor_copy(v_bf[:M, b * 17 + 16, :], stgv[:M, :])

                xt_r = d_xt.rearrange("(s p) t -> p s t", p=128)
                for tb in range(16):               # 256-token blocks, b-major
                    b = tb // 8
                    t0 = (tb % 8) * 256
                    xbf = xs.tile([128, 8, 256], bf16, tag="xbf")
                    nc.sync.dma_start(
                        xbf[:], xt_r[:, :, tb * 256:(tb + 1) * 256])
                    for sl in range(8):            # K^T slices
                        kps = psA.tile([128, 256], fp32, tag="mm")
                        for a in range(8):
                            nc.tensor.matmul(
                                kps[:], wkv_bf[:, a, sl * 128:(sl + 1) * 128],
                                xbf[:, a, :], start=(a == 0), stop=(a == 7))
                        nc.scalar.copy(
                            kt_bf[:, sl, b * WB + t0:b * WB + t0 + 256], kps[:])
                    for sub in range(2):           # V 128-row blocks
                        blk = (t0 + sub * 128) // 128
                        for half in range(2):
                            vps = psB.tile([128, 512], fp32, tag="mm2")
                            for a in range(8):
                                nc.tensor.matmul(
                                    vps[:], xbf[:, a, sub * 128:(sub + 1) * 128],
                                    wkv_bf[:, a, DIM + half * 512:DIM + (half + 1) * 512],
                                    start=(a == 0), stop=(a == 7))
                            nc.scalar.copy(
                                v_bf[:, b * 17 + blk, half * 512:(half + 1) * 512],
                                vps[:])

            # ---------- stage 2: q^T ----------
            with tc.tile_pool(name="wB", bufs=1) as wB:
                wq_bf = wB.tile([128, 8, DIM], bf16, tag="wq8")
                nc.sync.dma_start(
                    wq_bf[:], d_wq.rearrange("(s p) t -> p s t", p=128))
                xtq_bf = wB.tile([128, 8, 512], bf16, tag="xtq")
                nc.sync.dma_start(
                    xtq_bf[:], d_xtq.rearrange("(s p) t -> p s t", p=128))
                for sl in range(8):
                    qps = psA.tile([128, 512], fp32, tag="mm")
                    for a in range(8):
                        nc.tensor.matmul(
                            qps[:], wq_bf[:, a, sl * 128:(sl + 1) * 128],
                            xtq_bf[:, a, :], start=(a == 0), stop=(a == 7))
                    nc.scalar.copy(qt_bf[:, sl, :], qps[:])

            # ---------- stages 3-4 ----------
            with tc.tile_pool(name="late", bufs=1) as lp, \
                 tc.tile_pool(name="work", bufs=1) as wp, \
                 tc.tile_pool(name="qpp", bufs=2) as qpp, \
                 tc.tile_pool(name="tiny", bufs=6) as smp, \
                 tc.tile_pool(name="trp", bufs=4) as trp:
                wo_bf = lp.tile([128, 8, DIM], bf16, tag="wo")
                nc.sync.dma_start(
                    wo_bf[:], d_wo.rearrange("(s p) t -> p s t", p=128))
                mask_a = lp.tile([128, WA], bf16, tag="ma")
                mask_b = lp.tile([128, WB], bf16, tag="mb")
                nc.sync.dma_start(mask_a[:], d_mA[:])
                nc.sync.dma_start(mask_b[:], d_mB[:])
                bo_bf = lp.tile([128, DIM], bf16, tag="bo")
                nc.sync.dma_start(bo_bf[:], d_bo[:])
                sq_f = lp.tile([128, 8 * H], fp32, tag="sq")
                nc.sync.dma_start(sq_f[:], d_sq[:])
                outT = lp.tile([128, 4 * 8, 128], bf16, tag="outT")

                oT_pair = None
                for b in range(2):
                    for g in range(H):
                        qp = qpp.tile([128, 8, 256], bf16, tag="qp")
                        for sl in range(8):
                            nc.scalar.activation(
                                qp[:, sl, :], qt_bf[:, sl, b * 256:(b + 1) * 256],
                                ActF.Copy,
                                scale=sq_f[:, sl * H + g:sl * H + g + 1])
                        dots = wp.tile([128, WA + WB], bf16, tag="dots")
                        ebuf = wp.tile([128, WA + WB], bf16, tag="ebuf")
                        # (dst j0, kt src col, width) blocks; tile A's last
                        # block maps to the mem+pad block at kt cols [2048,2176)
                        blocks_a = [(0, 0, 512), (512, 512, 512), (1024, 2048, 128)]
                        blocks_b = [(j0, j0, min(512, WB - j0))
                                    for j0 in range(0, WB, 512)]
                        for t, (qc, off, msk, blks) in enumerate(
                                [(0, 0, mask_a, blocks_a),
                                 (128, WA, mask_b, blocks_b)]):
                            for (j0, src, jw) in blks:
                                dps = psA.tile([128, 512], fp32, tag="mm")
                                for a in range(8):
                                    nc.tensor.matmul(
                                        dps[:, :jw], qp[:, a, qc:qc + 128],
                                        kt_bf[:, a, b * WB + src:b * WB + src + jw],
                                        start=(a == 0), stop=(a == 7))
                                nc.vector.tensor_tensor(
                                    dots[:, off + j0:off + j0 + jw], dps[:, :jw],
                                    msk[:, j0:j0 + jw], Alu.add)
                        # threshold search
                        lo2 = smp.tile([128, 2], fp32, tag="lo2")
                        cn2 = smp.tile([128, 2], fp32, tag="cn2")
                        id2 = smp.tile([128, 2], fp32, tag="id2")
                        t2 = smp.tile([128, 2], fp32, tag="t2")
                        nc.vector.memset(lo2[:], BRLO)
                        w = BRW
                        for it in range(0 if 'nosearch' in DBG else NSEARCH):
                            w *= 0.5
                            nc.vector.tensor_scalar(t2[:], lo2[:], w, None, Alu.add)
                            nc.vector.tensor_scalar(
                                ebuf[:, :WA], dots[:, :WA], t2[:, 0:1], None,
                                Alu.is_ge, Alu.add,
                                accum_out=cn2[:, 0:1])
                            nc.vector.tensor_scalar(
                                ebuf[:, WA:], dots[:, WA:], t2[:, 1:2], None,
                                Alu.is_ge, Alu.add,
                                accum_out=cn2[:, 1:2])
                            nc.vector.tensor_scalar(
                                id2[:], cn2[:], float(TOPK), w, Alu.is_ge, Alu.mult)
                            nc.vector.tensor_tensor(lo2[:], lo2[:], id2[:], Alu.add)
                        nt2 = smp.tile([128, 2], fp32, tag="nt2")
                        nc.vector.tensor_scalar(nt2[:], lo2[:], -1.0, None, Alu.mult)
                        z2 = smp.tile([128, 2], fp32, tag="z2")
                        rz2 = smp.tile([128, 2], fp32, tag="rz2")
                        for t, (W, off) in enumerate([(WA, 0), (WB, WA)]):
                            sl_ = slice(off, off + W)
                            if 'noexp' not in DBG:
                                nc.scalar.activation(
                                    ebuf[:, sl_], dots[:, sl_], ActF.Exp,
                                    bias=(0.0 if 'nobias' in DBG else nt2[:, t:t + 1]),
                                    scale=1.0)
                            if 'nostt' not in DBG:
                                nc.vector.scalar_tensor_tensor(
                                    dots[:, sl_], ebuf[:, sl_], 1.0, ebuf[:, sl_],
                                    Alu.is_ge, Alu.mult, accum_out=z2[:, t:t + 1])
                        nc.vector.reciprocal(rz2[:], z2[:])
                        for t, (W, off) in enumerate([(WA, 0), (WB, WA)]):
                            sl_ = slice(off, off + W)
                            nc.vector.tensor_scalar(
                                ebuf[:, sl_], dots[:, sl_], rz2[:, t:t + 1], None,
                                Alu.mult)
                        # AV
                        if g % 2 == 0:
                            oT_a = psO.tile([128, 128], fp32, tag="oTa")
                            oT_b = psO.tile([128, 128], fp32, tag="oTb")
                            oT_pair = (oT_a, oT_b)
                        for t, (W, off, nb, oT) in enumerate(
                                [(WA, 0, NBA, oT_pair[0]), (WB, WA, NBB, oT_pair[1])]):
                            for jb in range(nb):
                                vblk = jb
                                if t == 0 and jb == NBA - 1:
                                    vblk = 16      # tile A's last block is mem+pad
                                emt = trp.tile([128, 128], bf16, tag="emt")
                                nc.sync.dma_start_transpose(
                                    emt[:],
                                    ebuf[:, off + jb * 128:off + (jb + 1) * 128])
                                nc.tensor.matmul(
                                    oT[(g % 2) * 64:(g % 2) * 64 + 64, :],
                                    v_bf[:, b * 17 + vblk, g * 64:(g + 1) * 64],
                                    emt[:], start=(jb == 0), stop=(jb == nb - 1))
                        if g % 2 == 1:
                            for t in range(2):
                                nc.scalar.copy(
                                    outT[:, (b * 2 + t) * 8 + g // 2, :],
                                    oT_pair[t][:])

                for bt in range(4):
                    ysb = wp.tile([128, DIM], fp32, tag="ysb")
                    for half in range(2):
                        yps = psB.tile([128, 512], fp32, tag="mm2")
                        for sl in range(8):
                            nc.tensor.matmul(
                                yps[:], outT[:, bt * 8 + sl, :],
                                wo_bf[:, sl, half * 512:(half + 1) * 512],
                                start=(sl == 0), stop=(sl == 7))
                        nc.vector.tensor_tensor(
                            ysb[:, half * 512:(half + 1) * 512], yps[:],
                            bo_bf[:, half * 512:(half + 1) * 512], Alu.add)
                    nc.sync.dma_start(d_y[bt * 128:(bt + 1) * 128, :], ysb[:])

    nc.finalize()
    return nc


def _prepare_in_maps(inputs):
    x = np.asarray(inputs["x"], np.float32)
    Wq = np.asarray(inputs["Wq"], np.float32)
    Wkv = np.asarray(inputs["Wkv"], np.float32)
    Wo = np.asarray(inputs["Wo"], np.float32)
    bo = np.asarray(inputs["bo"], np.float32)
    pre = np.asarray(inputs["pre_proj"], np.float32)
    mem_k = np.asarray(inputs["mem_k"], np.float32)
    mem_v = np.asarray(inputs["mem_v"], np.float32)

    xt_all = np.ascontiguousarray(
        np.concatenate([x[0].T, x[1].T], axis=1)).astype(BF)
    sq = np.empty((128, 8 * H), np.float32)
    for sl in range(8):
        for p in range(128):
            h = (sl * 128 + p) // DH
            sq[p, sl * H:(sl + 1) * H] = pre[h, :] * SCALE
    bob = np.broadcast_to(bo, (128, DIM)).astype(BF).copy()
    memkT = np.ascontiguousarray(
        mem_k.transpose(0, 2, 1).reshape(H * DH, M)).astype(BF)
    memv = np.ascontiguousarray(
        mem_v.transpose(1, 0, 2).reshape(M, H * DH)).astype(BF)
    wq_b, wkv_b, wo_b = Wq.astype(BF), Wkv.astype(BF), Wo.astype(BF)

    in_maps = []
    for c in range(8):
        tlo, thi = c, 15 - c
        rows_lo = np.arange(tlo * 128, tlo * 128 + 128)
        rows_hi = np.arange(thi * 128, thi * 128 + 128)
        cols = []
        for b in range(B):
            cols.append(x[b][rows_lo].T)
            cols.append(x[b][rows_hi].T)
        xtq = np.ascontiguousarray(np.concatenate(cols, axis=1)).astype(BF)

        def mk_mask(rows, W):
            m = np.full((128, W), NEGF, np.float32)
            for p, i in enumerate(rows):
                m[p, :min(i + 1, N)] = 0.0
                if W > N:
                    m[p, N:N + M] = 0.0      # tile B: mem at [2048,2064)
                else:
                    m[p, min(i + 1, 1024):] = NEGF
                    m[p, 1024:1024 + M] = 0.0  # tile A: mem block remapped here
            return m.astype(BF)
        in_maps.append({
            "xtq": xtq, "xt": xt_all, "wq": wq_b, "wkv": wkv_b, "wo": wo_b,
            "bob": bob, "sq": sq, "maska": mk_mask(rows_lo, WA),
            "maskb": mk_mask(rows_hi, WB), "memkt": memkT, "memv": memv,
        })
    return in_maps


def kernel(**inputs):
    from concourse import bass_utils
    if "nc" not in _CACHE:
        _CACHE["nc"] = _build_nc()
    nc = _CACHE["nc"]
    in_maps = _prepare_in_maps(inputs)
    res = bass_utils.run_bass_kernel_spmd(nc, in_maps, core_ids=list(range(8)))
    outs = res.results
    y = np.empty((B, N, DIM), np.float32)
    for c in range(8):
        yc = outs[c]["y"]
        tlo, thi = c, 15 - c
        for b in range(B):
            y[b, tlo * 128:(tlo + 1) * 128] = yc[(b * 2) * 128:(b * 2 + 1) * 128]
            y[b, thi * 128:(thi + 1) * 128] = yc[(b * 2 + 1) * 128:(b * 2 + 2) * 128]
    return y



# revision 16
# speedup vs baseline: 2.4123x; 2.4123x over previous
"""Trainium2 Bass kernel for sparse (top-k=64) talking-heads causal attention.

Sharding (batch x query-block slots): core c owns batch c%2 and its query
blocks {c//2, 4+c//2, 8+c//2, 12+c//2} (one per "slot" k=0..3). Slot k's
key range is the fixed prefix of (4k+4) data blocks + the 16 memory keys,
identical on every core; the true causal boundary (which depends on c//2)
is enforced by per-core additive masks (host data). Talking-heads mixing
couples all 16 heads at fixed (b,i,j), so full rows stay core-local.
Per-core work is identical by construction: 44 key-blocks across the 4
slots, and each core computes K/V for only its own batch.

Talking-heads fold: mixed[b,g,i,j] = sum_{h,d} (pre[h,g]*SCALE*q[b,h,i,d]) * K[b,j,(h,d)]
 -> one 1024-contraction matmul per output head g with per-head-scaled q'.
Memory k/v occupy kt cols [2048,2064) (j order is irrelevant: top-k /
softmax / AV are permutation invariant).

Top-64 threshold: 12-step binary search on exact counts, engine-split per
slot to balance load: slots 0/1 count on GpSimd (fused is_ge+accum), slot 2
on the DVE (fused, 1x mode), slot 3 as a DVE 4x-mode indicator summed on
the Activation engine (Copy activation with accum_out). Rows with <= 64
valid entries converge to t=-16 => keep all.

Perf structure: software-pipelined per-g loop — QK matmuls for g+1 are
emitted ahead of the search for g; PSUM->SBUF mask-evictions for g+1 land
between the search and softmax of g in the DVE FIFO; attention-weight
transposes are single batched xbar issues per slot on the Sync HW-DGE
queue.
"""
import os
import sys
import numpy as np
import ml_dtypes

sys.path.insert(0, "/opt/trn_rl_repo")

B, N, DIM = 2, 2048, 1024
H, DH = 16, 64
M = 16
TOPK = 64
SCALE = DH ** -0.5
NEGF = -3.0e38
BF = ml_dtypes.bfloat16

KTW = 17 * 128                      # kt cols: 2048 data + 16 mem + 112 pad
DATA_W = [512, 1024, 1536, 2048]    # slot data widths
SW = [w + 128 for w in DATA_W]      # slot total widths (mem+pad block last)
OFF = [0, 640, 1792, 3456]          # slot offsets in dots
WTOT = 5632
NB = [w // 128 for w in SW]         # 5, 9, 13, 17
NSEARCH = int(os.environ.get("KNS", "12"))
BRLO, BRW = -16.0, 32.0
# per-slot count engine: 'gp' = GpSimd fused, 'dve' = DVE fused,
# 'split' = DVE 4x indicator + ActE accumulate
CNT_ENG = os.environ.get("KCNT", "dve,split,dve,split").split(",")

_CACHE = {}
DBG = set(os.environ.get("KDBG", "").split(","))


def _build_nc():
    import concourse.mybir as mybir
    from concourse import bacc, tile

    fp32 = mybir.dt.float32
    bf16 = mybir.dt.bfloat16
    Alu = mybir.AluOpType
    ActF = mybir.ActivationFunctionType

    nc = bacc.Bacc(None, target_bir_lowering=False)

    d_xtq = nc.dram_tensor("xtq", [DIM, 512], bf16, kind="ExternalInput")
    d_xt = nc.dram_tensor("xt", [DIM, N], bf16, kind="ExternalInput")
    d_wq = nc.dram_tensor("wq", [DIM, DIM], bf16, kind="ExternalInput")
    d_wkv = nc.dram_tensor("wkv", [DIM, 2 * DIM], bf16, kind="ExternalInput")
    d_wo = nc.dram_tensor("wo", [DIM, DIM], bf16, kind="ExternalInput")
    d_bo = nc.dram_tensor("bob", [128, DIM], bf16, kind="ExternalInput")
    d_sq = nc.dram_tensor("sq", [128, 8 * H], fp32, kind="ExternalInput")
    d_mask = nc.dram_tensor("mask", [128, WTOT], bf16, kind="ExternalInput")
    d_mkT = nc.dram_tensor("memkt", [DIM, M], bf16, kind="ExternalInput")
    d_mv = nc.dram_tensor("memv", [M, DIM], bf16, kind="ExternalInput")
    d_y = nc.dram_tensor("y", [512, DIM], fp32, kind="ExternalOutput")

    with tile.TileContext(nc) as tc:
        with tc.tile_pool(name="persist", bufs=1) as pp, \
             tc.tile_pool(name="psA", bufs=2, space="PSUM") as psA, \
             tc.tile_pool(name="psB", bufs=2, space="PSUM") as psB, \
             tc.tile_pool(name="psO", bufs=2, space="PSUM") as psO:

            kt_bf = pp.tile([128, 8, KTW], bf16, tag="kt")
            v_bf = pp.tile([128, 17, DIM], bf16, tag="v")   # 16 data + mem
            qt_bf = pp.tile([128, 8, 512], bf16, tag="qt")
            outT = pp.tile([128, 4 * 8, 128], bf16, tag="outT")

            nc.vector.memset(kt_bf[:], 0.0)
            nc.vector.memset(v_bf[:, 16, :], 0.0)

            # ---------- stages 0-1: K^T, V (own batch only) ----------
            with tc.tile_pool(name="wA", bufs=1) as wA, \
                 tc.tile_pool(name="xs", bufs=2) as xs:
                wkv_bf = wA.tile([128, 8, 2 * DIM], bf16, tag="wkv")
                nc.sync.dma_start(
                    wkv_bf[:], d_wkv.rearrange("(s p) t -> p s t", p=128))
                # mem keys / values
                stgk = wA.tile([128, 8 * M], bf16, tag="stgk")
                for s in range(8):
                    nc.sync.dma_start(
                        stgk[:, s * M:(s + 1) * M], d_mkT[s * 128:(s + 1) * 128, :])
                for s in range(8):
                    nc.vector.tensor_copy(
                        kt_bf[:, s, N:N + M], stgk[:, s * M:(s + 1) * M])
                stgv = wA.tile([128, DIM], bf16, tag="stgv")
                nc.sync.dma_start(stgv[:M, :], d_mv[:, :])
                nc.vector.tensor_copy(v_bf[:M, 16, :], stgv[:M, :])

                xt_r = d_xt.rearrange("(s p) t -> p s t", p=128)
                for tb in range(8):                # 256-token blocks
                    t0 = tb * 256
                    xbf = xs.tile([128, 8, 256], bf16, tag="xbf")
                    nc.sync.dma_start(
                        xbf[:], xt_r[:, :, t0:t0 + 256])
                    for sl in range(8):            # K^T slices
                        kps = psA.tile([128, 256], fp32, tag="mm")
                        for a in range(8):
                            nc.tensor.matmul(
                                kps[:], wkv_bf[:, a, sl * 128:(sl + 1) * 128],
                                xbf[:, a, :], start=(a == 0), stop=(a == 7))
                        nc.scalar.copy(kt_bf[:, sl, t0:t0 + 256], kps[:])
                    for sub in range(2):           # V 128-row blocks
                        blk = t0 // 128 + sub
                        for half in range(2):
                            vps = psB.tile([128, 512], fp32, tag="mm2")
                            for a in range(8):
                                nc.tensor.matmul(
                                    vps[:], xbf[:, a, sub * 128:(sub + 1) * 128],
                                    wkv_bf[:, a, DIM + half * 512:DIM + (half + 1) * 512],
                                    start=(a == 0), stop=(a == 7))
                            nc.scalar.copy(
                                v_bf[:, blk, half * 512:(half + 1) * 512],
                                vps[:])

            # ---------- stage 2: q^T ----------
            with tc.tile_pool(name="wB", bufs=1) as wB:
                wq_bf = wB.tile([128, 8, DIM], bf16, tag="wq8")
                nc.sync.dma_start(
                    wq_bf[:], d_wq.rearrange("(s p) t -> p s t", p=128))
                xtq_bf = wB.tile([128, 8, 512], bf16, tag="xtq")
                nc.sync.dma_start(
                    xtq_bf[:], d_xtq.rearrange("(s p) t -> p s t", p=128))
                for sl in range(8):
                    qps = psA.tile([128, 512], fp32, tag="mm")
                    for a in range(8):
                        nc.tensor.matmul(
                            qps[:], wq_bf[:, a, sl * 128:(sl + 1) * 128],
                            xtq_bf[:, a, :], start=(a == 0), stop=(a == 7))
                    nc.scalar.copy(qt_bf[:, sl, :], qps[:])

            # ---------- stages 3-4 ----------
            with tc.tile_pool(name="late", bufs=1) as lp, \
                 tc.tile_pool(name="work", bufs=2) as wp, \
                 tc.tile_pool(name="qpp", bufs=2) as qpp, \
                 tc.tile_pool(name="tiny", bufs=3) as smp, \
                 tc.tile_pool(name="trp", bufs=2) as trp:
                mask_t = lp.tile([128, WTOT], bf16, tag="maskt")
                nc.sync.dma_start(mask_t[:], d_mask[:])
                sq_f = lp.tile([128, 8 * H], fp32, tag="sq")
                nc.sync.dma_start(sq_f[:], d_sq[:])

                # per-slot (dots offset, kt src, width) chunks; mem last
                SCHUNKS = []
                for k in range(4):
                    ch = [(OFF[k] + j0, j0, 512)
                          for j0 in range(0, DATA_W[k], 512)]
                    ch.append((OFF[k] + DATA_W[k], N, 128))
                    SCHUNKS.append(ch)

                def emit_qp_qk(g):
                    """ActE: scaled q' for head g; PE: QK matmuls into psA."""
                    qp = qpp.tile([128, 8, 512], bf16, tag="qp")
                    for sl in range(8):
                        nc.scalar.activation(
                            qp[:, sl, :], qt_bf[:, sl, :], ActF.Copy,
                            scale=sq_f[:, sl * H + g:sl * H + g + 1])
                    tiles = []
                    for k in range(4):
                        for (doff, src, jw) in SCHUNKS[k]:
                            dps = psA.tile([128, 512], fp32, tag="mm")
                            for a in range(8):
                                nc.tensor.matmul(
                                    dps[:, :jw],
                                    qp[:, a, k * 128:(k + 1) * 128],
                                    kt_bf[:, a, src:src + jw],
                                    start=(a == 0), stop=(a == 7))
                            tiles.append(dps)
                    return tiles

                def emit_maskadd(g, qk_tiles):
                    """DVE: evict psA -> dots with additive causal mask."""
                    dots = wp.tile([128, WTOT], bf16, tag="dots")
                    ti = 0
                    for k in range(4):
                        for (doff, src, jw) in SCHUNKS[k]:
                            nc.vector.tensor_tensor(
                                dots[:, doff:doff + jw],
                                qk_tiles[ti][:, :jw],
                                mask_t[:, doff:doff + jw], Alu.add)
                            ti += 1
                    return dots

                def emit_search(dots):
                    """12-step binary search for the per-row top-64 threshold.
                    Counting split across GpSimd / DVE / ActE per CNT_ENG."""
                    ebuf = wp.tile([128, WTOT], bf16, tag="ebuf")
                    t4 = smp.tile([128, 4], fp32, tag="t4")
                    cn4 = smp.tile([128, 4], fp32, tag="cn4")
                    id4 = smp.tile([128, 4], fp32, tag="id4")
                    lo4 = smp.tile([128, 4], fp32, tag="lo4")
                    nt4 = smp.tile([128, 4], fp32, tag="nt4")
                    nc.vector.memset(t4[:], BRLO + BRW * 0.5)
                    w = BRW * 0.5
                    for it in range(0 if 'nosearch' in DBG else NSEARCH):
                        # DVE indicators for 'split' slots first so ActE can
                        # start summing while DVE does its fused slots
                        for k in range(4):
                            if CNT_ENG[k] != 'split':
                                continue
                            sl_ = slice(OFF[k], OFF[k] + SW[k])
                            nc.vector.tensor_scalar(
                                ebuf[:, sl_], dots[:, sl_],
                                t4[:, k:k + 1], None, Alu.is_ge)
                            nc.scalar.activation(
                                ebuf[:, sl_], ebuf[:, sl_], ActF.Copy,
                                accum_out=cn4[:, k:k + 1])
                        for k in range(4):
                            if CNT_ENG[k] == 'split':
                                continue
                            eng = nc.gpsimd if CNT_ENG[k] == 'gp' else nc.vector
                            sl_ = slice(OFF[k], OFF[k] + SW[k])
                            eng.tensor_scalar(
                                ebuf[:, sl_], dots[:, sl_],
                                t4[:, k:k + 1], None, Alu.is_ge, Alu.add,
                                accum_out=cn4[:, k:k + 1])
                        last = (it == NSEARCH - 1)
                        # id4 = (cnt >= 64) * w
                        nc.vector.tensor_scalar(
                            id4[:], cn4[:], float(TOPK), w, Alu.is_ge, Alu.mult)
                        if not last:
                            # t = t + id4 - w/2  (next probe)
                            nc.vector.scalar_tensor_tensor(
                                t4[:], id4[:], -0.5 * w, t4[:], Alu.add, Alu.add)
                            w *= 0.5
                        else:
                            # lo = t + id4 - w  (last verified-ge threshold)
                            nc.vector.scalar_tensor_tensor(
                                lo4[:], id4[:], -w, t4[:], Alu.add, Alu.add)
                    if 'nosearch' in DBG:
                        nc.vector.memset(lo4[:], BRLO)
                    nc.vector.tensor_scalar(nt4[:], lo4[:], -1.0, None, Alu.mult)
                    return ebuf, nt4

                def emit_softmax(dots, ebuf, nt4):
                    """ActE exp; DVE mask+denominator+normalize."""
                    z4 = smp.tile([128, 4], fp32, tag="z4")
                    rz4 = smp.tile([128, 4], fp32, tag="rz4")
                    for k in range(4):
                        sl_ = slice(OFF[k], OFF[k] + SW[k])
                        if 'noexp' not in DBG:
                            nc.scalar.activation(
                                ebuf[:, sl_], dots[:, sl_], ActF.Exp,
                                bias=(0.0 if 'nobias' in DBG else nt4[:, k:k + 1]),
                                scale=1.0)
                        if 'nostt' not in DBG:
                            nc.vector.scalar_tensor_tensor(
                                dots[:, sl_], ebuf[:, sl_], 1.0, ebuf[:, sl_],
                                Alu.is_ge, Alu.mult, accum_out=z4[:, k:k + 1])
                    nc.vector.reciprocal(rz4[:], z4[:])
                    for k in range(4):
                        sl_ = slice(OFF[k], OFF[k] + SW[k])
                        nc.vector.tensor_scalar(
                            ebuf[:, sl_], dots[:, sl_], rz4[:, k:k + 1], None,
                            Alu.mult)
                    return ebuf

                def emit_av(g, ebuf, oT4):
                    """Batched xbar transposes (one per slot, Sync HW-DGE)
                    + PE AV accumulation. oT4 is one [128, 4, 128] PSUM
                    tile (one bank) holding all 4 slots' accumulators."""
                    for k in range(4):
                        emt = trp.tile([128, NB[k], 128], bf16, tag=f"emt{k}")
                        nc.sync.dma_start_transpose(
                            emt[:], ebuf[:, OFF[k]:OFF[k] + SW[k]])
                        for jb in range(NB[k]):
                            vblk = 16 if jb == NB[k] - 1 else jb
                            nc.tensor.matmul(
                                oT4[(g % 2) * 64:(g % 2) * 64 + 64, k, :],
                                v_bf[:, vblk, g * 64:(g + 1) * 64],
                                emt[:, jb, :],
                                start=(jb == 0), stop=(jb == NB[k] - 1))

                # prologue: head 0
                qk_tiles = emit_qp_qk(0)
                dots = emit_maskadd(0, qk_tiles)
                oT4 = None
                for g in range(H):
                    if g + 1 < H:
                        qk_next = emit_qp_qk(g + 1)
                    ebuf, nt4 = emit_search(dots)
                    if g + 1 < H:
                        dots_next = emit_maskadd(g + 1, qk_next)
                    ebuf = emit_softmax(dots, ebuf, nt4)
                    if g % 2 == 0:
                        oT4 = psO.tile([128, 4, 128], fp32, tag="oT4")
                    emit_av(g, ebuf, oT4)
                    if g % 2 == 1:
                        for k in range(4):
                            nc.scalar.copy(
                                outT[:, k * 8 + g // 2, :], oT4[:, k, :])
                    if g + 1 < H:
                        dots = dots_next

            # ---------- stage 5: output projection ----------
            with tc.tile_pool(name="tail", bufs=1) as tl, \
                 tc.tile_pool(name="ysp", bufs=2) as ysp:
                wo_bf = tl.tile([128, 8, DIM], bf16, tag="wo")
                nc.sync.dma_start(
                    wo_bf[:], d_wo.rearrange("(s p) t -> p s t", p=128))
                bo_bf = tl.tile([128, DIM], bf16, tag="bo")
                nc.sync.dma_start(bo_bf[:], d_bo[:])
                for bt in range(4):
                    ysb = ysp.tile([128, DIM], fp32, tag="ysb")
                    for half in range(2):
                        yps = psB.tile([128, 512], fp32, tag="mm2")
                        for sl in range(8):
                            nc.tensor.matmul(
                                yps[:], outT[:, bt * 8 + sl, :],
                                wo_bf[:, sl, half * 512:(half + 1) * 512],
                                start=(sl == 0), stop=(sl == 7))
                        nc.vector.tensor_tensor(
                            ysb[:, half * 512:(half + 1) * 512], yps[:],
                            bo_bf[:, half * 512:(half + 1) * 512], Alu.add)
                    nc.sync.dma_start(d_y[bt * 128:(bt + 1) * 128, :], ysb[:])

    nc.finalize()
    return nc


def _prepare_in_maps(inputs):
    x = np.asarray(inputs["x"], np.float32)
    Wq = np.asarray(inputs["Wq"], np.float32)
    Wkv = np.asarray(inputs["Wkv"], np.float32)
    Wo = np.asarray(inputs["Wo"], np.float32)
    bo = np.asarray(inputs["bo"], np.float32)
    pre = np.asarray(inputs["pre_proj"], np.float32)
    mem_k = np.asarray(inputs["mem_k"], np.float32)
    mem_v = np.asarray(inputs["mem_v"], np.float32)

    sq = np.empty((128, 8 * H), np.float32)
    for sl in range(8):
        for p in range(128):
            h = (sl * 128 + p) // DH
            sq[p, sl * H:(sl + 1) * H] = pre[h, :] * SCALE
    bob = np.broadcast_to(bo, (128, DIM)).astype(BF).copy()
    memkT = np.ascontiguousarray(
        mem_k.transpose(0, 2, 1).reshape(H * DH, M)).astype(BF)
    memv = np.ascontiguousarray(
        mem_v.transpose(1, 0, 2).reshape(M, H * DH)).astype(BF)
    wq_b, wkv_b, wo_b = Wq.astype(BF), Wkv.astype(BF), Wo.astype(BF)
    xt_b = [np.ascontiguousarray(x[b].T).astype(BF) for b in range(B)]

    in_maps = []
    for c in range(8):
        bc = c % 2
        tq = c // 2
        tks = [4 * k + tq for k in range(4)]
        cols = [x[bc][tk * 128:(tk + 1) * 128].T for tk in tks]
        xtq = np.ascontiguousarray(np.concatenate(cols, axis=1)).astype(BF)

        m = np.full((128, WTOT), NEGF, np.float32)
        for k, tk in enumerate(tks):
            for p in range(128):
                i = tk * 128 + p
                m[p, OFF[k]:OFF[k] + min(i + 1, DATA_W[k])] = 0.0
                m[p, OFF[k] + DATA_W[k]:OFF[k] + DATA_W[k] + M] = 0.0
        in_maps.append({
            "xtq": xtq, "xt": xt_b[bc], "wq": wq_b, "wkv": wkv_b,
            "wo": wo_b, "bob": bob, "sq": sq, "mask": m.astype(BF),
            "memkt": memkT, "memv": memv,
        })
    return in_maps


def kernel(**inputs):
    from concourse import bass_utils
    if "nc" not in _CACHE:
        _CACHE["nc"] = _build_nc()
    nc = _CACHE["nc"]
    in_maps = _prepare_in_maps(inputs)
    res = bass_utils.run_bass_kernel_spmd(nc, in_maps, core_ids=list(range(8)))
    outs = res.results
    y = np.empty((B, N, DIM), np.float32)
    for c in range(8):
        yc = outs[c]["y"]
        bc = c % 2
        tq = c // 2
        for k in range(4):
            tk = 4 * k + tq
            y[bc, tk * 128:(tk + 1) * 128] = yc[k * 128:(k + 1) * 128]
    return y


# revision 17
# speedup vs baseline: 2.9242x; 1.2122x over previous
"""Trainium2 Bass kernel for sparse (top-k=64) talking-heads causal attention.

Sharding (batch x query-block slots): core c owns batch c%2 and its query
blocks {c//2, 4+c//2, 8+c//2, 12+c//2} (one per "slot" k=0..3). Slot k's
key range is the fixed prefix of (4k+4) data blocks + the 16 memory keys,
identical on every core; the true causal boundary (which depends on c//2)
is enforced by per-core additive masks (host data). Talking-heads mixing
couples all 16 heads at fixed (b,i,j), so full rows stay core-local.
Per-core work is identical by construction: 44 key-blocks across the 4
slots, and each core computes K/V for only its own batch.

Talking-heads fold: mixed[b,g,i,j] = sum_{h,d} (pre[h,g]*SCALE*q[b,h,i,d]) * K[b,j,(h,d)]
 -> one 1024-contraction matmul per output head g with per-head-scaled q'.
Memory k/v occupy kt cols [2048,2064) (j order is irrelevant: top-k /
softmax / AV are permutation invariant).

Top-64 threshold: 12-step binary search on exact counts, engine-split per
slot to balance load: slots 0/1 count on GpSimd (fused is_ge+accum), slot 2
on the DVE (fused, 1x mode), slot 3 as a DVE 4x-mode indicator summed on
the Activation engine (Copy activation with accum_out). Rows with <= 64
valid entries converge to t=-16 => keep all.

Perf structure: software-pipelined per-g loop — QK matmuls for g+1 are
emitted ahead of the search for g; PSUM->SBUF mask-evictions for g+1 land
between the search and softmax of g in the DVE FIFO; attention-weight
transposes are single batched xbar issues per slot on the Sync HW-DGE
queue.
"""
import os
import sys
import numpy as np
import ml_dtypes

sys.path.insert(0, "/opt/trn_rl_repo")

B, N, DIM = 2, 2048, 1024
H, DH = 16, 64
M = 16
TOPK = 64
SCALE = DH ** -0.5
NEGF = -3.0e38
BF = ml_dtypes.bfloat16

KTW = 17 * 128                      # kt cols: 2048 data + 16 mem + 112 pad
DATA_W = [512, 1024, 1536, 2048]    # slot data widths
SW = [w + 128 for w in DATA_W]      # slot total widths (mem+pad block last)
OFF = [0, 640, 1792, 3456]          # slot offsets in dots
WTOT = 5632
NB = [w // 128 for w in SW]         # 5, 9, 13, 17
NSEARCH = int(os.environ.get("KNS", "10"))
BRLO, BRW = -16.0, 32.0
# per-slot count engine: 'gp' = GpSimd fused, 'dve' = DVE fused,
# 'split' = DVE 4x indicator + ActE accumulate
CNT_ENG = os.environ.get("KCNT", "dve,split,dve,split").split(",")

_CACHE = {}
DBG = set(os.environ.get("KDBG", "").split(","))


def _build_nc():
    import concourse.mybir as mybir
    from concourse import bacc, tile

    fp32 = mybir.dt.float32
    bf16 = mybir.dt.bfloat16
    Alu = mybir.AluOpType
    ActF = mybir.ActivationFunctionType

    nc = bacc.Bacc(None, target_bir_lowering=False)

    d_xtq = nc.dram_tensor("xtq", [DIM, 512], bf16, kind="ExternalInput")
    d_xt = nc.dram_tensor("xt", [DIM, N], bf16, kind="ExternalInput")
    d_wq = nc.dram_tensor("wq", [DIM, DIM], bf16, kind="ExternalInput")
    d_wkv = nc.dram_tensor("wkv", [DIM, 2 * DIM], bf16, kind="ExternalInput")
    d_wo = nc.dram_tensor("wo", [DIM, DIM], bf16, kind="ExternalInput")
    d_bo = nc.dram_tensor("bob", [128, DIM], bf16, kind="ExternalInput")
    d_sq = nc.dram_tensor("sq", [128, 8 * H], fp32, kind="ExternalInput")
    d_mask = nc.dram_tensor("mask", [128, WTOT], bf16, kind="ExternalInput")
    d_mkT = nc.dram_tensor("memkt", [DIM, M], bf16, kind="ExternalInput")
    d_mv = nc.dram_tensor("memv", [M, DIM], bf16, kind="ExternalInput")
    d_y = nc.dram_tensor("y", [512, DIM], fp32, kind="ExternalOutput")

    with tile.TileContext(nc) as tc:
        with tc.tile_pool(name="persist", bufs=1) as pp, \
             tc.tile_pool(name="psA", bufs=4, space="PSUM") as psA, \
             tc.tile_pool(name="psB", bufs=2, space="PSUM") as psB, \
             tc.tile_pool(name="psO", bufs=2, space="PSUM") as psO:

            kt_bf = pp.tile([128, 8, KTW], bf16, tag="kt")
            v_bf = pp.tile([128, 17, DIM], bf16, tag="v")   # 16 data + mem
            qt_bf = pp.tile([128, 8, 512], bf16, tag="qt")
            outT = pp.tile([128, 4 * 8, 128], bf16, tag="outT")

            nc.vector.memset(kt_bf[:], 0.0)
            nc.vector.memset(v_bf[:, 16, :], 0.0)

            # ---------- stages 0-1: K^T, V (own batch only) ----------
            with tc.tile_pool(name="wA", bufs=1) as wA, \
                 tc.tile_pool(name="xs", bufs=2) as xs:
                wkv_bf = wA.tile([128, 8, 2 * DIM], bf16, tag="wkv")
                nc.sync.dma_start(
                    wkv_bf[:], d_wkv.rearrange("(s p) t -> p s t", p=128))
                # mem keys / values
                stgk = wA.tile([128, 8 * M], bf16, tag="stgk")
                for s in range(8):
                    nc.sync.dma_start(
                        stgk[:, s * M:(s + 1) * M], d_mkT[s * 128:(s + 1) * 128, :])
                for s in range(8):
                    nc.vector.tensor_copy(
                        kt_bf[:, s, N:N + M], stgk[:, s * M:(s + 1) * M])
                stgv = wA.tile([128, DIM], bf16, tag="stgv")
                nc.sync.dma_start(stgv[:M, :], d_mv[:, :])
                nc.vector.tensor_copy(v_bf[:M, 16, :], stgv[:M, :])

                xt_r = d_xt.rearrange("(s p) t -> p s t", p=128)
                for tb in range(8):                # 256-token blocks
                    t0 = tb * 256
                    xbf = xs.tile([128, 8, 256], bf16, tag="xbf")
                    nc.sync.dma_start(
                        xbf[:], xt_r[:, :, t0:t0 + 256])
                    for sl in range(8):            # K^T slices
                        kps = psA.tile([128, 256], fp32, tag="mm")
                        for a in range(8):
                            nc.tensor.matmul(
                                kps[:], wkv_bf[:, a, sl * 128:(sl + 1) * 128],
                                xbf[:, a, :], start=(a == 0), stop=(a == 7))
                        nc.scalar.copy(kt_bf[:, sl, t0:t0 + 256], kps[:])
                    for sub in range(2):           # V 128-row blocks
                        blk = t0 // 128 + sub
                        for half in range(2):
                            vps = psB.tile([128, 512], fp32, tag="mm2")
                            for a in range(8):
                                nc.tensor.matmul(
                                    vps[:], xbf[:, a, sub * 128:(sub + 1) * 128],
                                    wkv_bf[:, a, DIM + half * 512:DIM + (half + 1) * 512],
                                    start=(a == 0), stop=(a == 7))
                            nc.scalar.copy(
                                v_bf[:, blk, half * 512:(half + 1) * 512],
                                vps[:])

            # ---------- stage 2: q^T ----------
            with tc.tile_pool(name="wB", bufs=1) as wB:
                wq_bf = wB.tile([128, 8, DIM], bf16, tag="wq8")
                nc.sync.dma_start(
                    wq_bf[:], d_wq.rearrange("(s p) t -> p s t", p=128))
                xtq_bf = wB.tile([128, 8, 512], bf16, tag="xtq")
                nc.sync.dma_start(
                    xtq_bf[:], d_xtq.rearrange("(s p) t -> p s t", p=128))
                for sl in range(8):
                    qps = psA.tile([128, 512], fp32, tag="mm")
                    for a in range(8):
                        nc.tensor.matmul(
                            qps[:], wq_bf[:, a, sl * 128:(sl + 1) * 128],
                            xtq_bf[:, a, :], start=(a == 0), stop=(a == 7))
                    nc.scalar.copy(qt_bf[:, sl, :], qps[:])

            # ---------- stages 3-4 ----------
            with tc.tile_pool(name="late", bufs=1) as lp, \
                 tc.tile_pool(name="work", bufs=2) as wp, \
                 tc.tile_pool(name="qpp", bufs=2) as qpp, \
                 tc.tile_pool(name="tiny", bufs=3) as smp, \
                 tc.tile_pool(name="trp", bufs=2) as trp:
                mask_t = lp.tile([128, WTOT], bf16, tag="maskt")
                nc.sync.dma_start(mask_t[:], d_mask[:])
                sq_f = lp.tile([128, 8 * H], fp32, tag="sq")
                nc.sync.dma_start(sq_f[:], d_sq[:])

                # per-slot (dots offset, kt src, width) chunks; mem last
                SCHUNKS = []
                for k in range(4):
                    ch = [(OFF[k] + j0, j0, 512)
                          for j0 in range(0, DATA_W[k], 512)]
                    ch.append((OFF[k] + DATA_W[k], N, 128))
                    SCHUNKS.append(ch)

                def emit_qp_qk(g):
                    """ActE: scaled q' for head g; PE: QK matmuls into psA."""
                    qp = qpp.tile([128, 8, 512], bf16, tag="qp")
                    for sl in range(8):
                        nc.scalar.activation(
                            qp[:, sl, :], qt_bf[:, sl, :], ActF.Copy,
                            scale=sq_f[:, sl * H + g:sl * H + g + 1])
                    tiles = []
                    for k in range(4):
                        for (doff, src, jw) in SCHUNKS[k]:
                            dps = psA.tile([128, 512], fp32, tag="mm")
                            for a in range(8):
                                nc.tensor.matmul(
                                    dps[:, :jw],
                                    qp[:, a, k * 128:(k + 1) * 128],
                                    kt_bf[:, a, src:src + jw],
                                    start=(a == 0), stop=(a == 7))
                            tiles.append(dps)
                    return tiles

                def emit_maskadd(g, qk_tiles):
                    """DVE: evict psA -> dots with additive causal mask."""
                    dots = wp.tile([128, WTOT], bf16, tag="dots")
                    ti = 0
                    for k in range(4):
                        for (doff, src, jw) in SCHUNKS[k]:
                            nc.vector.tensor_tensor(
                                dots[:, doff:doff + jw],
                                qk_tiles[ti][:, :jw],
                                mask_t[:, doff:doff + jw], Alu.add)
                            ti += 1
                    return dots

                def emit_search(dots):
                    """12-step binary search for the per-row top-64 threshold.
                    Counting split across GpSimd / DVE / ActE per CNT_ENG."""
                    ebuf = wp.tile([128, WTOT], bf16, tag="ebuf")
                    t4 = smp.tile([128, 4], fp32, tag="t4")
                    cn4 = smp.tile([128, 4], fp32, tag="cn4")
                    id4 = smp.tile([128, 4], fp32, tag="id4")
                    lo4 = smp.tile([128, 4], fp32, tag="lo4")
                    nt4 = smp.tile([128, 4], fp32, tag="nt4")
                    nc.gpsimd.memset(t4[:], BRLO + BRW * 0.5)
                    w = BRW * 0.5
                    for it in range(0 if 'nosearch' in DBG else NSEARCH):
                        # DVE indicators for 'split' slots first so ActE can
                        # start summing while DVE does its fused slots
                        for k in range(4):
                            if CNT_ENG[k] != 'split':
                                continue
                            sl_ = slice(OFF[k], OFF[k] + SW[k])
                            nc.vector.tensor_scalar(
                                ebuf[:, sl_], dots[:, sl_],
                                t4[:, k:k + 1], None, Alu.is_ge)
                            nc.scalar.activation(
                                ebuf[:, sl_], ebuf[:, sl_], ActF.Copy,
                                accum_out=cn4[:, k:k + 1])
                        for k in range(4):
                            if CNT_ENG[k] == 'split':
                                continue
                            eng = nc.gpsimd if CNT_ENG[k] == 'gp' else nc.vector
                            sl_ = slice(OFF[k], OFF[k] + SW[k])
                            eng.tensor_scalar(
                                ebuf[:, sl_], dots[:, sl_],
                                t4[:, k:k + 1], None, Alu.is_ge, Alu.add,
                                accum_out=cn4[:, k:k + 1])
                        last = (it == NSEARCH - 1)
                        # small-op chain on GpSimd (frees the DVE):
                        # id4 = (cnt >= 64) * w; t/lo updates
                        nc.gpsimd.tensor_scalar(
                            id4[:], cn4[:], float(TOPK), w, Alu.is_ge, Alu.mult)
                        if not last:
                            # t = t + (id4 - w/2)  (next probe)
                            nc.gpsimd.tensor_scalar(
                                id4[:], id4[:], -0.5 * w, None, Alu.add)
                            nc.gpsimd.tensor_tensor(
                                t4[:], t4[:], id4[:], Alu.add)
                            w *= 0.5
                        else:
                            # lo = t + (id4 - w)  (last verified-ge threshold)
                            nc.gpsimd.tensor_scalar(
                                id4[:], id4[:], -w, None, Alu.add)
                            nc.gpsimd.tensor_tensor(
                                lo4[:], t4[:], id4[:], Alu.add)
                    if 'nosearch' in DBG:
                        nc.gpsimd.memset(lo4[:], BRLO)
                    nc.gpsimd.tensor_scalar(nt4[:], lo4[:], -1.0, None, Alu.mult)
                    return ebuf, nt4

                def emit_softmax(dots, ebuf, nt4):
                    """ActE exp; DVE mask+denominator+normalize."""
                    z4 = smp.tile([128, 4], fp32, tag="z4")
                    rz4 = smp.tile([128, 4], fp32, tag="rz4")
                    for k in range(4):
                        sl_ = slice(OFF[k], OFF[k] + SW[k])
                        if 'noexp' not in DBG:
                            nc.scalar.activation(
                                ebuf[:, sl_], dots[:, sl_], ActF.Exp,
                                bias=(0.0 if 'nobias' in DBG else nt4[:, k:k + 1]),
                                scale=1.0)
                        if 'nostt' not in DBG:
                            nc.vector.scalar_tensor_tensor(
                                dots[:, sl_], ebuf[:, sl_], 1.0, ebuf[:, sl_],
                                Alu.is_ge, Alu.mult, accum_out=z4[:, k:k + 1])
                    nc.vector.reciprocal(rz4[:], z4[:])
                    for k in range(4):
                        sl_ = slice(OFF[k], OFF[k] + SW[k])
                        nc.vector.tensor_scalar(
                            ebuf[:, sl_], dots[:, sl_], rz4[:, k:k + 1], None,
                            Alu.mult)
                    return ebuf

                def emit_av(g, ebuf, oT4):
                    """Batched xbar transposes (one per slot, Sync HW-DGE)
                    + PE AV accumulation. oT4 is one [128, 4, 128] PSUM
                    tile (one bank) holding all 4 slots' accumulators."""
                    for k in range(4):
                        emt = trp.tile([128, NB[k], 128], bf16, tag=f"emt{k}")
                        nc.sync.dma_start_transpose(
                            emt[:], ebuf[:, OFF[k]:OFF[k] + SW[k]])
                        for jb in range(NB[k]):
                            vblk = 16 if jb == NB[k] - 1 else jb
                            nc.tensor.matmul(
                                oT4[(g % 2) * 64:(g % 2) * 64 + 64, k, :],
                                v_bf[:, vblk, g * 64:(g + 1) * 64],
                                emt[:, jb, :],
                                start=(jb == 0), stop=(jb == NB[k] - 1))

                # prologue: head 0
                qk_tiles = emit_qp_qk(0)
                dots = emit_maskadd(0, qk_tiles)
                oT4 = None
                for g in range(H):
                    if g + 1 < H:
                        qk_next = emit_qp_qk(g + 1)
                    ebuf, nt4 = emit_search(dots)
                    if g + 1 < H:
                        dots_next = emit_maskadd(g + 1, qk_next)
                    ebuf = emit_softmax(dots, ebuf, nt4)
                    if g % 2 == 0:
                        oT4 = psO.tile([128, 4, 128], fp32, tag="oT4")
                    emit_av(g, ebuf, oT4)
                    if g % 2 == 1:
                        for k in range(4):
                            nc.scalar.copy(
                                outT[:, k * 8 + g // 2, :], oT4[:, k, :])
                    if g + 1 < H:
                        dots = dots_next

            # ---------- stage 5: output projection ----------
            with tc.tile_pool(name="tail", bufs=1) as tl, \
                 tc.tile_pool(name="ysp", bufs=2) as ysp:
                wo_bf = tl.tile([128, 8, DIM], bf16, tag="wo")
                nc.sync.dma_start(
                    wo_bf[:], d_wo.rearrange("(s p) t -> p s t", p=128))
                bo_bf = tl.tile([128, DIM], bf16, tag="bo")
                nc.sync.dma_start(bo_bf[:], d_bo[:])
                for bt in range(4):
                    ysb = ysp.tile([128, DIM], fp32, tag="ysb")
                    for half in range(2):
                        yps = psB.tile([128, 512], fp32, tag="mm2")
                        for sl in range(8):
                            nc.tensor.matmul(
                                yps[:], outT[:, bt * 8 + sl, :],
                                wo_bf[:, sl, half * 512:(half + 1) * 512],
                                start=(sl == 0), stop=(sl == 7))
                        nc.vector.tensor_tensor(
                            ysb[:, half * 512:(half + 1) * 512], yps[:],
                            bo_bf[:, half * 512:(half + 1) * 512], Alu.add)
                    nc.sync.dma_start(d_y[bt * 128:(bt + 1) * 128, :], ysb[:])

    nc.finalize()
    return nc


def _prepare_in_maps(inputs):
    x = np.asarray(inputs["x"], np.float32)
    Wq = np.asarray(inputs["Wq"], np.float32)
    Wkv = np.asarray(inputs["Wkv"], np.float32)
    Wo = np.asarray(inputs["Wo"], np.float32)
    bo = np.asarray(inputs["bo"], np.float32)
    pre = np.asarray(inputs["pre_proj"], np.float32)
    mem_k = np.asarray(inputs["mem_k"], np.float32)
    mem_v = np.asarray(inputs["mem_v"], np.float32)

    sq = np.empty((128, 8 * H), np.float32)
    for sl in range(8):
        for p in range(128):
            h = (sl * 128 + p) // DH
            sq[p, sl * H:(sl + 1) * H] = pre[h, :] * SCALE
    bob = np.broadcast_to(bo, (128, DIM)).astype(BF).copy()
    memkT = np.ascontiguousarray(
        mem_k.transpose(0, 2, 1).reshape(H * DH, M)).astype(BF)
    memv = np.ascontiguousarray(
        mem_v.transpose(1, 0, 2).reshape(M, H * DH)).astype(BF)
    wq_b, wkv_b, wo_b = Wq.astype(BF), Wkv.astype(BF), Wo.astype(BF)
    xt_b = [np.ascontiguousarray(x[b].T).astype(BF) for b in range(B)]

    in_maps = []
    for c in range(8):
        bc = c % 2
        tq = c // 2
        tks = [4 * k + tq for k in range(4)]
        cols = [x[bc][tk * 128:(tk + 1) * 128].T for tk in tks]
        xtq = np.ascontiguousarray(np.concatenate(cols, axis=1)).astype(BF)

        m = np.full((128, WTOT), NEGF, np.float32)
        for k, tk in enumerate(tks):
            for p in range(128):
                i = tk * 128 + p
                m[p, OFF[k]:OFF[k] + min(i + 1, DATA_W[k])] = 0.0
                m[p, OFF[k] + DATA_W[k]:OFF[k] + DATA_W[k] + M] = 0.0
        in_maps.append({
            "xtq": xtq, "xt": xt_b[bc], "wq": wq_b, "wkv": wkv_b,
            "wo": wo_b, "bob": bob, "sq": sq, "mask": m.astype(BF),
            "memkt": memkT, "memv": memv,
        })
    return in_maps


def kernel(**inputs):
    from concourse import bass_utils
    if "nc" not in _CACHE:
        _CACHE["nc"] = _build_nc()
    nc = _CACHE["nc"]
    in_maps = _prepare_in_maps(inputs)
    res = bass_utils.run_bass_kernel_spmd(nc, in_maps, core_ids=list(range(8)))
    outs = res.results
    y = np.empty((B, N, DIM), np.float32)
    for c in range(8):
        yc = outs[c]["y"]
        bc = c % 2
        tq = c // 2
        for k in range(4):
            tk = 4 * k + tq
            y[bc, tk * 128:(tk + 1) * 128] = yc[k * 128:(k + 1) * 128]
    return y


# revision 18
# speedup vs baseline: 2.9796x; 1.0189x over previous
"""Trainium2 Bass kernel for sparse (top-k=64) talking-heads causal attention.

Sharding (batch x query-block slots): core c owns batch c%2 and its query
blocks {c//2, 4+c//2, 8+c//2, 12+c//2} (one per "slot" k=0..3). Slot k's
key range is the fixed prefix of (4k+4) data blocks + the 16 memory keys,
identical on every core; the true causal boundary (which depends on c//2)
is enforced by per-core additive masks (host data). Talking-heads mixing
couples all 16 heads at fixed (b,i,j), so full rows stay core-local.
Per-core work is identical by construction: 44 key-blocks across the 4
slots, and each core computes K/V for only its own batch.

Talking-heads fold: mixed[b,g,i,j] = sum_{h,d} (pre[h,g]*SCALE*q[b,h,i,d]) * K[b,j,(h,d)]
 -> one 1024-contraction matmul per output head g with per-head-scaled q'.
Memory k/v occupy kt cols [2048,2064) (j order is irrelevant: top-k /
softmax / AV are permutation invariant).

Top-64 threshold: 12-step binary search on exact counts, engine-split per
slot to balance load: slots 0/1 count on GpSimd (fused is_ge+accum), slot 2
on the DVE (fused, 1x mode), slot 3 as a DVE 4x-mode indicator summed on
the Activation engine (Copy activation with accum_out). Rows with <= 64
valid entries converge to t=-16 => keep all.

Perf structure: software-pipelined per-g loop — QK matmuls for g+1 are
emitted ahead of the search for g; PSUM->SBUF mask-evictions for g+1 land
between the search and softmax of g in the DVE FIFO; attention-weight
transposes are single batched xbar issues per slot on the Sync HW-DGE
queue.
"""
import os
import sys
import numpy as np
import ml_dtypes

sys.path.insert(0, "/opt/trn_rl_repo")

B, N, DIM = 2, 2048, 1024
H, DH = 16, 64
M = 16
TOPK = 64
SCALE = DH ** -0.5
NEGF = -3.0e38
BF = ml_dtypes.bfloat16

KTW = 17 * 128                      # kt cols: 2048 data + 16 mem + 112 pad
DATA_W = [512, 1024, 1536, 2048]    # slot data widths
SW = [w + 128 for w in DATA_W]      # slot total widths (mem+pad block last)
OFF = [0, 640, 1792, 3456]          # slot offsets in dots
WTOT = 5632
NB = [w // 128 for w in SW]         # 5, 9, 13, 17
CW = [w + M for w in DATA_W]        # compute widths: data + 16 mem cols
NSEARCH = int(os.environ.get("KNS", "10"))
BRLO, BRW = -16.0, 32.0
# per-slot count engine: 'gp' = GpSimd fused, 'dve' = DVE fused,
# 'split' = DVE 4x indicator + ActE accumulate
CNT_ENG = os.environ.get("KCNT", "dve,split,dve,split").split(",")

_CACHE = {}
DBG = set(os.environ.get("KDBG", "").split(","))


def _build_nc():
    import concourse.mybir as mybir
    from concourse import bacc, tile

    fp32 = mybir.dt.float32
    bf16 = mybir.dt.bfloat16
    Alu = mybir.AluOpType
    ActF = mybir.ActivationFunctionType

    nc = bacc.Bacc(None, target_bir_lowering=False)

    d_xtq = nc.dram_tensor("xtq", [DIM, 512], bf16, kind="ExternalInput")
    d_xt = nc.dram_tensor("xt", [DIM, N], bf16, kind="ExternalInput")
    d_wq = nc.dram_tensor("wq", [DIM, DIM], bf16, kind="ExternalInput")
    d_wkv = nc.dram_tensor("wkv", [DIM, 2 * DIM], bf16, kind="ExternalInput")
    d_wo = nc.dram_tensor("wo", [DIM, DIM], bf16, kind="ExternalInput")
    d_bo = nc.dram_tensor("bob", [128, DIM], bf16, kind="ExternalInput")
    d_sq = nc.dram_tensor("sq", [128, 8 * H], fp32, kind="ExternalInput")
    d_mask = nc.dram_tensor("mask", [128, WTOT], bf16, kind="ExternalInput")
    d_eye = nc.dram_tensor("eye", [128, 128], bf16, kind="ExternalInput")
    d_mkT = nc.dram_tensor("memkt", [DIM, M], bf16, kind="ExternalInput")
    d_mv = nc.dram_tensor("memv", [M, DIM], bf16, kind="ExternalInput")
    d_y = nc.dram_tensor("y", [512, DIM], fp32, kind="ExternalOutput")

    with tile.TileContext(nc) as tc:
        with tc.tile_pool(name="persist", bufs=1) as pp, \
             tc.tile_pool(name="psA", bufs=4, space="PSUM") as psA, \
             tc.tile_pool(name="psB", bufs=2, space="PSUM") as psB, \
             tc.tile_pool(name="psO", bufs=2, space="PSUM") as psO:

            kt_bf = pp.tile([128, 8, KTW], bf16, tag="kt")
            v_bf = pp.tile([128, 17, DIM], bf16, tag="v")   # 16 data + mem
            qt_bf = pp.tile([128, 8, 512], bf16, tag="qt")
            outT = pp.tile([128, 4 * 8, 128], bf16, tag="outT")

            nc.vector.memset(kt_bf[:], 0.0)
            nc.vector.memset(v_bf[:, 16, :], 0.0)

            # ---------- stages 0-1: K^T, V (own batch only) ----------
            with tc.tile_pool(name="wA", bufs=1) as wA, \
                 tc.tile_pool(name="xs", bufs=2) as xs:
                wkv_bf = wA.tile([128, 8, 2 * DIM], bf16, tag="wkv")
                nc.sync.dma_start(
                    wkv_bf[:], d_wkv.rearrange("(s p) t -> p s t", p=128))
                # mem keys / values
                stgk = wA.tile([128, 8 * M], bf16, tag="stgk")
                for s in range(8):
                    nc.sync.dma_start(
                        stgk[:, s * M:(s + 1) * M], d_mkT[s * 128:(s + 1) * 128, :])
                for s in range(8):
                    nc.vector.tensor_copy(
                        kt_bf[:, s, N:N + M], stgk[:, s * M:(s + 1) * M])
                stgv = wA.tile([128, DIM], bf16, tag="stgv")
                nc.sync.dma_start(stgv[:M, :], d_mv[:, :])
                nc.vector.tensor_copy(v_bf[:M, 16, :], stgv[:M, :])

                xt_r = d_xt.rearrange("(s p) t -> p s t", p=128)
                for tb in range(8):                # 256-token blocks
                    t0 = tb * 256
                    xbf = xs.tile([128, 8, 256], bf16, tag="xbf")
                    nc.sync.dma_start(
                        xbf[:], xt_r[:, :, t0:t0 + 256])
                    for sl in range(8):            # K^T slices
                        kps = psA.tile([128, 256], fp32, tag="mm")
                        for a in range(8):
                            nc.tensor.matmul(
                                kps[:], wkv_bf[:, a, sl * 128:(sl + 1) * 128],
                                xbf[:, a, :], start=(a == 0), stop=(a == 7))
                        nc.scalar.copy(kt_bf[:, sl, t0:t0 + 256], kps[:])
                    for sub in range(2):           # V 128-row blocks
                        blk = t0 // 128 + sub
                        for half in range(2):
                            vps = psB.tile([128, 512], fp32, tag="mm2")
                            for a in range(8):
                                nc.tensor.matmul(
                                    vps[:], xbf[:, a, sub * 128:(sub + 1) * 128],
                                    wkv_bf[:, a, DIM + half * 512:DIM + (half + 1) * 512],
                                    start=(a == 0), stop=(a == 7))
                            nc.scalar.copy(
                                v_bf[:, blk, half * 512:(half + 1) * 512],
                                vps[:])

            # ---------- stage 2: q^T ----------
            with tc.tile_pool(name="wB", bufs=1) as wB:
                wq_bf = wB.tile([128, 8, DIM], bf16, tag="wq8")
                nc.sync.dma_start(
                    wq_bf[:], d_wq.rearrange("(s p) t -> p s t", p=128))
                xtq_bf = wB.tile([128, 8, 512], bf16, tag="xtq")
                nc.sync.dma_start(
                    xtq_bf[:], d_xtq.rearrange("(s p) t -> p s t", p=128))
                for sl in range(8):
                    qps = psA.tile([128, 512], fp32, tag="mm")
                    for a in range(8):
                        nc.tensor.matmul(
                            qps[:], wq_bf[:, a, sl * 128:(sl + 1) * 128],
                            xtq_bf[:, a, :], start=(a == 0), stop=(a == 7))
                    nc.scalar.copy(qt_bf[:, sl, :], qps[:])

            # ---------- stages 3-4 ----------
            with tc.tile_pool(name="late", bufs=1) as lp, \
                 tc.tile_pool(name="work", bufs=2) as wp, \
                 tc.tile_pool(name="qpp", bufs=2) as qpp, \
                 tc.tile_pool(name="tiny", bufs=3) as smp, \
                 tc.tile_pool(name="trp", bufs=2) as trp:
                mask_t = lp.tile([128, WTOT], bf16, tag="maskt")
                nc.sync.dma_start(mask_t[:], d_mask[:])
                eye_bf = lp.tile([128, 128], bf16, tag="eye")
                nc.sync.dma_start(eye_bf[:], d_eye[:])
                sq_f = lp.tile([128, 8 * H], fp32, tag="sq")
                nc.sync.dma_start(sq_f[:], d_sq[:])

                # per-slot (dots offset, kt src, width) chunks; mem last
                SCHUNKS = []
                for k in range(4):
                    ch = [(OFF[k] + j0, j0, 512)
                          for j0 in range(0, DATA_W[k], 512)]
                    ch.append((OFF[k] + DATA_W[k], N, M))
                    SCHUNKS.append(ch)

                def emit_qp_qk(g):
                    """q' for head g (split ActE/GpSimd); PE: QK matmuls
                    into psA with the additive causal mask folded in as an
                    identity-matmul accumulation (data chunks only — the
                    16 mem cols are always visible)."""
                    qp = qpp.tile([128, 8, 512], bf16, tag="qp")
                    for sl in range(8):
                        col = sl * H + g
                        if sl < 4:
                            nc.scalar.activation(
                                qp[:, sl, :], qt_bf[:, sl, :], ActF.Copy,
                                scale=sq_f[:, col:col + 1])
                        else:
                            nc.gpsimd.tensor_tensor(
                                qp[:, sl, :], qt_bf[:, sl, :],
                                sq_f[:, col:col + 1].broadcast_to([128, 512]),
                                Alu.mult)
                    tiles = []
                    for k in range(4):
                        for (doff, src, jw) in SCHUNKS[k]:
                            is_mem = (jw == M)
                            dps = psA.tile([128, 512], fp32, tag="mm")
                            for a in range(8):
                                nc.tensor.matmul(
                                    dps[:, :jw],
                                    qp[:, a, k * 128:(k + 1) * 128],
                                    kt_bf[:, a, src:src + jw],
                                    start=(a == 0),
                                    stop=(a == 7 and is_mem))
                            if not is_mem:
                                nc.tensor.matmul(
                                    dps[:, :jw], eye_bf[:],
                                    mask_t[:, doff:doff + jw],
                                    start=False, stop=True)
                            tiles.append(dps)
                    return tiles

                def emit_maskadd(g, qk_tiles):
                    """DVE: evict psA -> dots (mask already folded in)."""
                    dots = wp.tile([128, WTOT], bf16, tag="dots")
                    ti = 0
                    for k in range(4):
                        for (doff, src, jw) in SCHUNKS[k]:
                            nc.vector.tensor_copy(
                                dots[:, doff:doff + jw],
                                qk_tiles[ti][:, :jw])
                            ti += 1
                    return dots

                def emit_search(dots):
                    """12-step binary search for the per-row top-64 threshold.
                    Counting split across GpSimd / DVE / ActE per CNT_ENG."""
                    ebuf = wp.tile([128, WTOT], bf16, tag="ebuf")
                    for k in range(4):
                        # zero the 112 pad cols so the AV transpose sees 0
                        nc.gpsimd.memset(
                            ebuf[:, OFF[k] + CW[k]:OFF[k] + SW[k]], 0.0)
                    t4 = smp.tile([128, 4], fp32, tag="t4")
                    cn4 = smp.tile([128, 4], fp32, tag="cn4")
                    id4 = smp.tile([128, 4], fp32, tag="id4")
                    lo4 = smp.tile([128, 4], fp32, tag="lo4")
                    nt4 = smp.tile([128, 4], fp32, tag="nt4")
                    nc.gpsimd.memset(t4[:], BRLO + BRW * 0.5)
                    w = BRW * 0.5
                    for it in range(0 if 'nosearch' in DBG else NSEARCH):
                        # DVE indicators for 'split' slots first so ActE can
                        # start summing while DVE does its fused slots
                        for k in range(4):
                            if CNT_ENG[k] != 'split':
                                continue
                            sl_ = slice(OFF[k], OFF[k] + CW[k])
                            nc.vector.tensor_scalar(
                                ebuf[:, sl_], dots[:, sl_],
                                t4[:, k:k + 1], None, Alu.is_ge)
                            nc.scalar.activation(
                                ebuf[:, sl_], ebuf[:, sl_], ActF.Copy,
                                accum_out=cn4[:, k:k + 1])
                        for k in range(4):
                            if CNT_ENG[k] == 'split':
                                continue
                            eng = nc.gpsimd if CNT_ENG[k] == 'gp' else nc.vector
                            sl_ = slice(OFF[k], OFF[k] + CW[k])
                            eng.tensor_scalar(
                                ebuf[:, sl_], dots[:, sl_],
                                t4[:, k:k + 1], None, Alu.is_ge, Alu.add,
                                accum_out=cn4[:, k:k + 1])
                        last = (it == NSEARCH - 1)
                        # small-op chain on GpSimd (frees the DVE):
                        # id4 = (cnt >= 64) * w; t/lo updates
                        nc.gpsimd.tensor_scalar(
                            id4[:], cn4[:], float(TOPK), w, Alu.is_ge, Alu.mult)
                        if not last:
                            # t = t + (id4 - w/2)  (next probe)
                            nc.gpsimd.tensor_scalar(
                                id4[:], id4[:], -0.5 * w, None, Alu.add)
                            nc.gpsimd.tensor_tensor(
                                t4[:], t4[:], id4[:], Alu.add)
                            w *= 0.5
                        else:
                            # lo = t + (id4 - w)  (last verified-ge threshold)
                            nc.gpsimd.tensor_scalar(
                                id4[:], id4[:], -w, None, Alu.add)
                            nc.gpsimd.tensor_tensor(
                                lo4[:], t4[:], id4[:], Alu.add)
                    if 'nosearch' in DBG:
                        nc.gpsimd.memset(lo4[:], BRLO)
                    nc.gpsimd.tensor_scalar(nt4[:], lo4[:], -1.0, None, Alu.mult)
                    return ebuf, nt4

                def emit_softmax(dots, ebuf, nt4):
                    """ActE exp; DVE mask+denominator+normalize."""
                    z4 = smp.tile([128, 4], fp32, tag="z4")
                    rz4 = smp.tile([128, 4], fp32, tag="rz4")
                    for k in range(4):
                        sl_ = slice(OFF[k], OFF[k] + CW[k])
                        if 'noexp' not in DBG:
                            nc.scalar.activation(
                                ebuf[:, sl_], dots[:, sl_], ActF.Exp,
                                bias=(0.0 if 'nobias' in DBG else nt4[:, k:k + 1]),
                                scale=1.0)
                        if 'nostt' not in DBG:
                            nc.vector.scalar_tensor_tensor(
                                dots[:, sl_], ebuf[:, sl_], 1.0, ebuf[:, sl_],
                                Alu.is_ge, Alu.mult, accum_out=z4[:, k:k + 1])
                    nc.vector.reciprocal(rz4[:], z4[:])
                    for k in range(4):
                        sl_ = slice(OFF[k], OFF[k] + CW[k])
                        nc.vector.tensor_scalar(
                            ebuf[:, sl_], dots[:, sl_], rz4[:, k:k + 1], None,
                            Alu.mult)
                    return ebuf

                def emit_av(g, ebuf, oT4):
                    """Batched xbar transposes (one per slot, Sync HW-DGE)
                    + PE AV accumulation. oT4 is one [128, 4, 128] PSUM
                    tile (one bank) holding all 4 slots' accumulators."""
                    for k in range(4):
                        emt = trp.tile([128, NB[k], 128], bf16, tag=f"emt{k}")
                        nc.sync.dma_start_transpose(
                            emt[:], ebuf[:, OFF[k]:OFF[k] + SW[k]])
                        for jb in range(NB[k]):
                            vblk = 16 if jb == NB[k] - 1 else jb
                            nc.tensor.matmul(
                                oT4[(g % 2) * 64:(g % 2) * 64 + 64, k, :],
                                v_bf[:, vblk, g * 64:(g + 1) * 64],
                                emt[:, jb, :],
                                start=(jb == 0), stop=(jb == NB[k] - 1))

                # prologue: head 0
                qk_tiles = emit_qp_qk(0)
                dots = emit_maskadd(0, qk_tiles)
                oT4 = None
                for g in range(H):
                    if g + 1 < H:
                        qk_next = emit_qp_qk(g + 1)
                    ebuf, nt4 = emit_search(dots)
                    if g + 1 < H:
                        dots_next = emit_maskadd(g + 1, qk_next)
                    ebuf = emit_softmax(dots, ebuf, nt4)
                    if g % 2 == 0:
                        oT4 = psO.tile([128, 4, 128], fp32, tag="oT4")
                    emit_av(g, ebuf, oT4)
                    if g % 2 == 1:
                        for k in range(4):
                            nc.vector.tensor_copy(
                                outT[:, k * 8 + g // 2, :], oT4[:, k, :])
                    if g + 1 < H:
                        dots = dots_next

            # ---------- stage 5: output projection ----------
            with tc.tile_pool(name="tail", bufs=1) as tl, \
                 tc.tile_pool(name="ysp", bufs=2) as ysp:
                wo_bf = tl.tile([128, 8, DIM], bf16, tag="wo")
                nc.sync.dma_start(
                    wo_bf[:], d_wo.rearrange("(s p) t -> p s t", p=128))
                bo_bf = tl.tile([128, DIM], bf16, tag="bo")
                nc.sync.dma_start(bo_bf[:], d_bo[:])
                for bt in range(4):
                    ysb = ysp.tile([128, DIM], fp32, tag="ysb")
                    for half in range(2):
                        yps = psB.tile([128, 512], fp32, tag="mm2")
                        for sl in range(8):
                            nc.tensor.matmul(
                                yps[:], outT[:, bt * 8 + sl, :],
                                wo_bf[:, sl, half * 512:(half + 1) * 512],
                                start=(sl == 0), stop=(sl == 7))
                        nc.vector.tensor_tensor(
                            ysb[:, half * 512:(half + 1) * 512], yps[:],
                            bo_bf[:, half * 512:(half + 1) * 512], Alu.add)
                    nc.sync.dma_start(d_y[bt * 128:(bt + 1) * 128, :], ysb[:])

    nc.finalize()
    return nc


def _prepare_in_maps(inputs):
    x = np.asarray(inputs["x"], np.float32)
    Wq = np.asarray(inputs["Wq"], np.float32)
    Wkv = np.asarray(inputs["Wkv"], np.float32)
    Wo = np.asarray(inputs["Wo"], np.float32)
    bo = np.asarray(inputs["bo"], np.float32)
    pre = np.asarray(inputs["pre_proj"], np.float32)
    mem_k = np.asarray(inputs["mem_k"], np.float32)
    mem_v = np.asarray(inputs["mem_v"], np.float32)

    sq = np.empty((128, 8 * H), np.float32)
    for sl in range(8):
        for p in range(128):
            h = (sl * 128 + p) // DH
            sq[p, sl * H:(sl + 1) * H] = pre[h, :] * SCALE
    bob = np.broadcast_to(bo, (128, DIM)).astype(BF).copy()
    memkT = np.ascontiguousarray(
        mem_k.transpose(0, 2, 1).reshape(H * DH, M)).astype(BF)
    memv = np.ascontiguousarray(
        mem_v.transpose(1, 0, 2).reshape(M, H * DH)).astype(BF)
    wq_b, wkv_b, wo_b = Wq.astype(BF), Wkv.astype(BF), Wo.astype(BF)
    xt_b = [np.ascontiguousarray(x[b].T).astype(BF) for b in range(B)]

    in_maps = []
    for c in range(8):
        bc = c % 2
        tq = c // 2
        tks = [4 * k + tq for k in range(4)]
        cols = [x[bc][tk * 128:(tk + 1) * 128].T for tk in tks]
        xtq = np.ascontiguousarray(np.concatenate(cols, axis=1)).astype(BF)

        m = np.full((128, WTOT), NEGF, np.float32)
        for k, tk in enumerate(tks):
            for p in range(128):
                i = tk * 128 + p
                m[p, OFF[k]:OFF[k] + min(i + 1, DATA_W[k])] = 0.0
                m[p, OFF[k] + DATA_W[k]:OFF[k] + DATA_W[k] + M] = 0.0
        in_maps.append({
            "xtq": xtq, "xt": xt_b[bc], "wq": wq_b, "wkv": wkv_b,
            "wo": wo_b, "bob": bob, "sq": sq, "mask": m.astype(BF),
            "eye": np.eye(128, dtype=np.float32).astype(BF),
            "memkt": memkT, "memv": memv,
        })
    return in_maps


def kernel(**inputs):
    from concourse import bass_utils
    if "nc" not in _CACHE:
        _CACHE["nc"] = _build_nc()
    nc = _CACHE["nc"]
    in_maps = _prepare_in_maps(inputs)
    res = bass_utils.run_bass_kernel_spmd(nc, in_maps, core_ids=list(range(8)))
    outs = res.results
    y = np.empty((B, N, DIM), np.float32)
    for c in range(8):
        yc = outs[c]["y"]
        bc = c % 2
        tq = c // 2
        for k in range(4):
            tk = 4 * k + tq
            y[bc, tk * 128:(tk + 1) * 128] = yc[k * 128:(k + 1) * 128]
    return y
